# revision 5
# baseline (speedup 1.0000x reference)
"""GAT message-passing kernel for 8 Trainium2 NeuronCores.

Key algebraic property of the reference (faithful torch repeat_interleave
replication): with h = x @ proj_w.T + proj_b  [B, N, H],
    first[b, I, J, c]  = h[b, I, J*H+c) // N] = h[b, I, J // (N//H)]
    second[b, I, J, c] = h[b, I, c]
so the pre-mask score collapses to
    scores[b, I, J] = leaky_relu(S1 * h[b, I, J//32] + d[b, I])
with S1 = sum(a_w[0, :H]) and d = h @ a_w[0, H:].  Each row of scores has
only H=32 distinct values (one per 32-column block of J).  Softmax+matmul
then reduce to a masked weighted aggregation that never materializes any
[N, N] tensor in HBM:
    W[b, I, J] = adj[I, J] * exp(leaky(v))[b, I, J//32]
    out[b, I, :] = (W @ h[b]) / rowsum(W)

Sharding: rows I are split 128-per-core across 8 cores (both batches on
every core).  dist_mat rows are sharded; x and the tiny weights are
replicated; every core redundantly computes full h (trivial FLOPs).
"""

import sys

sys.path.insert(0, "/opt/trn_rl_repo")

import numpy as np

B, N, C, H = 2, 1024, 64, 32
P = 128                 # rows per core / partition tile
NCORES = 8
NT = B * N // P         # 16 token tiles of 128
NJ = N // P             # 8 column tiles of 128
THR = 200000.0
ALPHA = 0.01

_CACHE = {}
LAST_RESULT = None


def _build():
    import concourse.bacc as bacc
    import concourse.tile as tile
    from concourse import masks, mybir

    F32 = mybir.dt.float32
    Alu = mybir.AluOpType
    Act = mybir.ActivationFunctionType

    nc = bacc.Bacc("TRN2", target_bir_lowering=False)

    xg_d = nc.dram_tensor("xg", (NT, P, C), F32, kind="ExternalInput")
    xo_d = nc.dram_tensor("xo", (B, P, C), F32, kind="ExternalInput")
    dist_d = nc.dram_tensor("dist", (P, N), F32, kind="ExternalInput")
    wt_d = nc.dram_tensor("wt", (C + 1, H + 1), F32, kind="ExternalInput")
    m32_d = nc.dram_tensor("m32", (H, H), F32, kind="ExternalInput")
    ind_d = nc.dram_tensor("ind", (H, NJ, P), F32, kind="ExternalInput")
    out_d = nc.dram_tensor("out", (B, P, H), F32, kind="ExternalOutput")

    with tile.TileContext(nc) as tc:
        with (
            tc.tile_pool(name="const", bufs=1) as const,
            tc.tile_pool(name="persist", bufs=1) as persist,
            tc.tile_pool(name="work", bufs=3) as work,
            tc.tile_pool(name="psT", bufs=3, space="PSUM") as psT,
            tc.tile_pool(name="psE", bufs=2, space="PSUM") as psE,
            tc.tile_pool(name="psG", bufs=2, space="PSUM") as psG,
        ):
            ident = const.tile([P, P], F32)
            masks.make_identity(nc, ident[:])
            wt = const.tile([C + 1, H + 1], F32)
            nc.sync.dma_start(out=wt[:], in_=wt_d[:])
            m32 = const.tile([H, H], F32)
            nc.sync.dma_start(out=m32[:], in_=m32_d[:])
            ind = const.tile([H, NJ, P], F32)
            nc.sync.dma_start(out=ind[:], in_=ind_d[:])

            xg_sb = persist.tile([P, NT, C], F32)
            nc.sync.dma_start(out=xg_sb[:], in_=xg_d[:].rearrange("g p c -> p g c"))
            xo_sb = persist.tile([P, B, C], F32)
            nc.sync.dma_start(out=xo_sb[:], in_=xo_d[:].rearrange("b p c -> p b c"))
            dist_sb = persist.tile([P, NJ, P], F32)
            nc.sync.dma_start(
                out=dist_sb[:], in_=dist_d[:].rearrange("p (j q) -> p j q", j=NJ)
            )

            h_all = persist.tile([P, NT, H + 1], F32)
            adjT = persist.tile([P, NJ, P], F32)

            # ---- h (augmented with ones column) for all 16 token tiles ----
            for g in range(NT):
                ps_x = psT.tile([C, P], F32, tag="ps")
                nc.tensor.transpose(ps_x[:], xg_sb[:, g, :], ident[:])
                xta = work.tile([C + 1, P], F32, tag="xta")
                nc.vector.tensor_copy(xta[0:C, :], ps_x[:])
                nc.gpsimd.memset(xta[C : C + 1, :], 1.0)
                ps_h = psT.tile([P, H + 1], F32, tag="ps")
                nc.tensor.matmul(ps_h[:], xta[:], wt[:])
                nc.scalar.copy(h_all[:, g, :], ps_h[:])

            # ---- adjacency (transposed), shared across both batches ----
            for j in range(NJ):
                ps_d = psT.tile([P, P], F32, tag="ps")
                nc.tensor.transpose(ps_d[:], dist_sb[:, j, :], ident[:])
                nc.vector.tensor_scalar(
                    out=adjT[:, j, :], in0=ps_d[:],
                    scalar1=THR, scalar2=None, op0=Alu.is_lt,
                )

            # ---- per batch: scores -> masked weighted aggregation ----
            for b in range(B):
                ps_xo = psT.tile([C, P], F32, tag="ps")
                nc.tensor.transpose(ps_xo[:], xo_sb[:, b, :], ident[:])
                xoa = work.tile([C + 1, P], F32, tag="xta")
                nc.vector.tensor_copy(xoa[0:C, :], ps_xo[:])
                nc.gpsimd.memset(xoa[C : C + 1, :], 1.0)
                ps_hT = psT.tile([H + 1, P], F32, tag="ps")
                nc.tensor.matmul(ps_hT[:], wt[:], xoa[:])
                hT = work.tile([H, P], F32, tag="hT")
                nc.vector.tensor_copy(hT[:], ps_hT[0:H, :])

                ps_v = psT.tile([H, P], F32, tag="ps")
                nc.tensor.matmul(ps_v[:], m32[:], hT[:])
                t1 = work.tile([H, P], F32, tag="t1")
                nc.vector.tensor_scalar(
                    out=t1[:], in0=ps_v[:], scalar1=ALPHA, scalar2=None, op0=Alu.mult
                )
                t2 = work.tile([H, P], F32, tag="t2")
                nc.vector.tensor_tensor(out=t2[:], in0=ps_v[:], in1=t1[:], op=Alu.max)
                eT = work.tile([H, P], F32, tag="eT")
                nc.scalar.activation(eT[:], t2[:], Act.Exp)

                ps_agg = psG.tile([H + 1, P], F32)
                for j in range(NJ):
                    ps_eb = psE.tile([P, P], F32)
                    nc.tensor.matmul(ps_eb[:], ind[:, j, :], eT[:])
                    wtile = work.tile([P, P], F32, tag="wtile")
                    nc.vector.tensor_mul(wtile[:], adjT[:, j, :], ps_eb[:])
                    nc.tensor.matmul(
                        ps_agg[:], h_all[:, b * NJ + j, :], wtile[:],
                        start=(j == 0), stop=(j == NJ - 1),
                    )

                outT = work.tile([H + 1, P], F32, tag="outT")
                nc.scalar.copy(outT[:], ps_agg[:])
                ps_on = psT.tile([P, H + 1], F32, tag="ps")
                nc.tensor.transpose(ps_on[:], outT[:], ident[0 : H + 1, 0 : H + 1])
                zr = work.tile([P, 1], F32, tag="zr")
                nc.vector.reciprocal(zr[:], ps_on[:, H : H + 1])
                ot = work.tile([P, H], F32, tag="ot")
                nc.vector.tensor_scalar_mul(out=ot[:], in0=ps_on[:, 0:H], scalar1=zr[:])
                nc.sync.dma_start(out=out_d[b], in_=ot[:])

    nc.finalize()
    return nc


def kernel(x, dist_mat, proj_w, proj_b, a_w, trace=False):
    global LAST_RESULT
    from concourse.bass_utils import run_bass_kernel_spmd

    x = np.ascontiguousarray(np.asarray(x, dtype=np.float32))
    dist_mat = np.asarray(dist_mat, dtype=np.float32)
    proj_w = np.asarray(proj_w, dtype=np.float32)
    proj_b = np.asarray(proj_b, dtype=np.float32).reshape(H)
    a_w = np.asarray(a_w, dtype=np.float32).reshape(2 * H)

    if "nc" not in _CACHE:
        _CACHE["nc"] = _build()
    nc = _CACHE["nc"]

    # host-side constant folding (all tiny)
    a1, a2 = a_w[:H], a_w[H:]
    s1 = np.float32(a1.sum(dtype=np.float32))
    m32 = s1 * np.eye(H, dtype=np.float32) + a2[:, None] * np.ones(
        (1, H), np.float32
    )
    wt = np.zeros((C + 1, H + 1), np.float32)
    wt[:C, :H] = proj_w.T
    wt[C, :H] = proj_b
    wt[C, H] = 1.0
    # ind[k, j, t] = 1 iff k == 4*j + t//32, so ind[:, j, :].T @ eT
    # broadcasts e-row k=(128*j+t)//32 across each 32-row partition group.
    ind = np.zeros((H, NJ, P), np.float32)
    for j in range(NJ):
        for g in range(4):
            ind[4 * j + g, j, 32 * g : 32 * (g + 1)] = 1.0

    dist_fixed = dist_mat.copy()
    np.fill_diagonal(dist_fixed, 0.0)  # adj diagonal forced to 1

    xg = x.reshape(NT, P, C)
    in_maps = []
    for c in range(NCORES):
        sl = slice(c * P, (c + 1) * P)
        in_maps.append(
            {
                "xg": xg,
                "xo": np.ascontiguousarray(x[:, sl, :]),
                "dist": dist_fixed[sl],
                "wt": wt,
                "m32": m32,
                "ind": ind,
            }
        )

    res = run_bass_kernel_spmd(nc, in_maps, core_ids=list(range(NCORES)), trace=trace)
    LAST_RESULT = res
    return np.concatenate([res.results[c]["out"] for c in range(NCORES)], axis=1)


# revision 6
# speedup vs baseline: 1.7933x; 1.7933x over previous
"""GAT message-passing kernel for 8 Trainium2 NeuronCores.

Key algebraic property of the reference (faithful torch repeat_interleave
replication): with h = x @ proj_w.T + proj_b  [B, N, H],
    first[b, I, J, c]  = h[b, I, (J*H+c) // N] = h[b, I, J // (N//H)]
    second[b, I, J, c] = h[b, I, c]
so the pre-mask score collapses to
    scores[b, I, J] = leaky_relu(S1 * h[b, I, J//32] + d[b, I])
with S1 = sum(a_w[0, :H]) and d = h @ a_w[0, H:].  Each row of scores has
only H=32 distinct values (one per 32-column block of J).  Softmax+matmul
then reduce to a masked weighted aggregation that never materializes any
[N, N] tensor in HBM:
    W[b, I, J] = adj[I, J] * exp(leaky(v))[b, I, J//32]
    out[b, I, :] = (W @ h[b]) / rowsum(W)

Sharding: rows I are split 128-per-core across 8 cores (both batches on
every core).  dist_mat rows are sharded; x and the tiny weights are
replicated; every core redundantly computes full h (trivial FLOPs).

Device dataflow per core (c = core index, I in [128c, 128c+128)):
  - h_aug[J, m] for all 16 token tiles via paired PE transposes of x and
    block-diagonal weight matmuls (2 tiles per matmul), bias added during
    the PSUM->SBUF copy against a broadcast bias tile.
  - adjT[J, I] = (distT < thr) via paired PE transposes of the core's
    dist rows (diagonal pre-zeroed on host so the forced diag-1 holds).
  - eT[k, t] = exp(leaky(M32a.T @ hT_aug)) in one small matmul chain,
    then spilled to DRAM and broadcast-replicated back into
    eb[J, I] = e[J//32, I] with strided replicate DMAs.
  - W.T tile = adjT * eb (DVE), aggregated with PSUM-accumulated
    matmuls: out_psum[I, m] += W.T_j.T @ h_aug_j; column 32 of h_aug is
    ones so out_psum[:, 32] = Z (softmax denominator).  Final divide is
    a per-partition reciprocal+scale, DMA'd straight out.
"""

import sys

sys.path.insert(0, "/opt/trn_rl_repo")

import numpy as np

B, N, C, H = 2, 1024, 64, 32
P = 128                 # rows per core / partition tile
NCORES = 8
NT = B * N // P         # 16 token tiles of 128
NJ = N // P             # 8 column tiles of 128
NPAIR = NT // 2         # 8 paired token tiles
THR = 200000.0
ALPHA = 0.01
H1 = H + 1              # 33: h channels + ones column
H2 = 2 * H1             # 66: two tiles side by side

_CACHE = {}
LAST_RESULT = None


def _build():
    import concourse.bacc as bacc
    import concourse.bass as bass
    import concourse.tile as tile
    from concourse import masks, mybir

    F32 = mybir.dt.float32
    Alu = mybir.AluOpType
    Act = mybir.ActivationFunctionType

    nc = bacc.Bacc("TRN2", target_bir_lowering=False)

    xg_d = nc.dram_tensor("xg", (NT, P, C), F32, kind="ExternalInput")
    xo_d = nc.dram_tensor("xo", (B, P, C), F32, kind="ExternalInput")
    dist_d = nc.dram_tensor("dist", (P, N), F32, kind="ExternalInput")
    wt2_d = nc.dram_tensor("wt2", (P, H2), F32, kind="ExternalInput")
    brow2_d = nc.dram_tensor("brow2", (1, H2), F32, kind="ExternalInput")
    wta_d = nc.dram_tensor("wta", (C + 1, H1), F32, kind="ExternalInput")
    m32a_d = nc.dram_tensor("m32a", (H1, H), F32, kind="ExternalInput")
    out_d = nc.dram_tensor("out", (B, P, H), F32, kind="ExternalOutput")

    with tile.TileContext(nc) as tc:
        with (
            tc.tile_pool(name="const", bufs=1) as const,
            tc.tile_pool(name="persist", bufs=1) as persist,
            tc.tile_pool(name="work", bufs=3) as work,
            tc.tile_pool(name="dram", bufs=1, space="DRAM") as drampool,
            tc.tile_pool(name="psT", bufs=4, space="PSUM") as psT,
            tc.tile_pool(name="psA", bufs=2, space="PSUM") as psA,
        ):
            # ---- input DMAs (issued up front, spread across both rings) ----
            xo_sb = persist.tile([P, B, C], F32)
            nc.sync.dma_start(out=xo_sb[:], in_=xo_d[:].rearrange("b p c -> p b c"))
            wt2 = const.tile([P, H2], F32)
            nc.scalar.dma_start(out=wt2[:], in_=wt2_d[:])
            wta = const.tile([C + 1, H1], F32)
            nc.scalar.dma_start(out=wta[:], in_=wta_d[:])
            m32a = const.tile([H1, H], F32)
            nc.scalar.dma_start(out=m32a[:], in_=m32a_d[:])
            bias2 = const.tile([P, H2], F32)
            nc.scalar.dma_start(
                out=bias2[:],
                in_=bass.AP(tensor=brow2_d, offset=0, ap=[[0, P], [1, H2]]),
            )
            dist_sb = persist.tile([P, NJ, P], F32)
            dview = dist_d[:].rearrange("p (j q) -> p j q", j=NJ)
            nc.sync.dma_start(out=dist_sb[:, 0:4, :], in_=dview[:, 0:4, :])
            nc.scalar.dma_start(out=dist_sb[:, 4:8, :], in_=dview[:, 4:8, :])
            xg_sb = persist.tile([P, NT, C], F32)
            xview = xg_d[:].rearrange("g p c -> p g c")
            for q in range(4):
                eng = nc.sync if q % 2 == 0 else nc.scalar
                eng.dma_start(
                    out=xg_sb[:, 4 * q : 4 * q + 4, :], in_=xview[:, 4 * q : 4 * q + 4, :]
                )

            ident = const.tile([P, P], F32)
            masks.make_identity(nc, ident[:])

            h_all = persist.tile([P, NPAIR, H2], F32)
            adjT = persist.tile([P, NJ, P], F32)
            esc = drampool.tile([B, H, P], F32)
            eb_all = persist.tile([P, B, NJ, P], F32)

            # ---- per batch: score factors e[k, t] ----
            for b in range(B):
                ps_xo = psT.tile([C, P], F32, tag="ps")
                nc.tensor.transpose(ps_xo[:], xo_sb[:, b, :], ident[:])
                xoa = work.tile([C + 1, P], F32, tag="xoa")
                nc.vector.tensor_copy(xoa[0:C, :], ps_xo[:])
                nc.gpsimd.memset(xoa[C : C + 1, :], 1.0)
                ps_hT = psT.tile([H1, P], F32, tag="ps")
                nc.tensor.matmul(ps_hT[:], wta[:], xoa[:])
                hToa = work.tile([H1, P], F32, tag="hToa")
                nc.vector.tensor_copy(hToa[:], ps_hT[:])
                ps_v = psT.tile([H, P], F32, tag="ps")
                nc.tensor.matmul(ps_v[:], m32a[:], hToa[:])
                t1 = work.tile([H, P], F32, tag="t1")
                nc.vector.tensor_scalar(
                    out=t1[:], in0=ps_v[:], scalar1=ALPHA, scalar2=None, op0=Alu.mult
                )
                t2 = work.tile([H, P], F32, tag="t2")
                nc.vector.tensor_tensor(out=t2[:], in0=ps_v[:], in1=t1[:], op=Alu.max)
                eT = work.tile([H, P], F32, tag="eT")
                nc.scalar.activation(eT[:], t2[:], Act.Exp)
                # spill to DRAM, then replicate rows 4j+g -> partitions 32g..
                nc.sync.dma_start(out=esc[b], in_=eT[:])
                for j in range(NJ):
                    src = esc[b, 4 * j : 4 * j + 4, :]
                    rep = bass.AP(
                        tensor=src.tensor,
                        offset=src.offset,
                        ap=[list(src.ap[0]), [0, H], list(src.ap[1])],
                    )
                    eng = nc.sync if j % 2 == 0 else nc.scalar
                    eng.dma_start(out=eb_all[:, b, j, :], in_=rep)

            # ---- h (aug) for all tokens: paired transpose + blockdiag matmul ----
            for p in range(NPAIR):
                ps_x = psT.tile([P, P], F32, tag="ps")
                nc.tensor.transpose(ps_x[:], xg_sb[:, 2 * p : 2 * p + 2, :], ident[:])
                xt2 = work.tile([P, P], F32, tag="xt2")
                nc.vector.tensor_copy(xt2[:], ps_x[:])
                ps_h = psT.tile([P, H2], F32, tag="ps")
                nc.tensor.matmul(ps_h[:], xt2[:], wt2[:])
                nc.vector.tensor_add(out=h_all[:, p, :], in0=ps_h[:], in1=bias2[:])

            # ---- adjacency (transposed), paired compare ----
            for jp in range(NJ // 2):
                ps_d = psT.tile([P, 2, P], F32, tag="ps")
                nc.tensor.transpose(ps_d[:, 0, :], dist_sb[:, 2 * jp, :], ident[:])
                nc.tensor.transpose(ps_d[:, 1, :], dist_sb[:, 2 * jp + 1, :], ident[:])
                nc.vector.tensor_scalar(
                    out=adjT[:, 2 * jp : 2 * jp + 2, :], in0=ps_d[:],
                    scalar1=THR, scalar2=None, op0=Alu.is_lt,
                )

            # ---- masked weighted aggregation ----
            for b in range(B):
                ps_o = psA.tile([P, H1], F32)
                for j in range(NJ):
                    wtile = work.tile([P, P], F32, tag="wtile")
                    nc.vector.tensor_mul(
                        wtile[:], adjT[:, j, :], eb_all[:, b, j, :]
                    )
                    g = b * NJ + j
                    nc.tensor.matmul(
                        ps_o[:],
                        wtile[:],
                        h_all[:, g // 2, H1 * (g % 2) : H1 * (g % 2) + H1],
                        start=(j == 0),
                        stop=(j == NJ - 1),
                    )
                zr = work.tile([P, 1], F32, tag="zr")
                nc.vector.reciprocal(zr[:], ps_o[:, H : H + 1])
                ot = work.tile([P, H], F32, tag="ot")
                nc.vector.tensor_scalar_mul(out=ot[:], in0=ps_o[:, 0:H], scalar1=zr[:])
                eng = nc.sync if b == 0 else nc.scalar
                eng.dma_start(out=out_d[b], in_=ot[:])

    nc.finalize()
    return nc


def kernel(x, dist_mat, proj_w, proj_b, a_w, trace=False):
    global LAST_RESULT
    from concourse.bass_utils import run_bass_kernel_spmd

    x = np.ascontiguousarray(np.asarray(x, dtype=np.float32))
    dist_mat = np.asarray(dist_mat, dtype=np.float32)
    proj_w = np.asarray(proj_w, dtype=np.float32)
    proj_b = np.asarray(proj_b, dtype=np.float32).reshape(H)
    a_w = np.asarray(a_w, dtype=np.float32).reshape(2 * H)

    if "nc" not in _CACHE:
        _CACHE["nc"] = _build()
    nc = _CACHE["nc"]

    # ---- host-side constant folding (all tiny) ----
    a1, a2 = a_w[:H], a_w[H:]
    s1 = np.float32(a1.sum(dtype=np.float32))
    # wT with a zero ones-column slot (col H), no bias row
    wt_nb = np.zeros((C, H1), np.float32)
    wt_nb[:, :H] = proj_w.T
    # block-diagonal weights: two token tiles per matmul
    wt2 = np.zeros((P, H2), np.float32)
    wt2[0:C, 0:H1] = wt_nb
    wt2[C:P, H1:H2] = wt_nb
    # broadcast bias row (+1.0 in the ones-column slots)
    b_aug = np.concatenate([proj_b, np.float32([1.0])])
    brow2 = np.concatenate([b_aug, b_aug]).reshape(1, H2)
    # hT path: weights with bias row and ones column
    wta = np.zeros((C + 1, H1), np.float32)
    wta[:C, :H] = proj_w.T
    wta[C, :H] = proj_b
    wta[C, H] = 1.0
    # v = S1*h + (h@a2): fold into one [33, 32] matrix against h_aug.T
    m32a = np.zeros((H1, H), np.float32)
    m32a[:H, :] = s1 * np.eye(H, dtype=np.float32) + a2[:, None]

    dist_fixed = dist_mat.copy()
    np.fill_diagonal(dist_fixed, 0.0)  # adj diagonal forced to 1

    xg = x.reshape(NT, P, C)
    in_maps = []
    for c in range(NCORES):
        sl = slice(c * P, (c + 1) * P)
        in_maps.append(
            {
                "xg": xg,
                "xo": np.ascontiguousarray(x[:, sl, :]),
                "dist": dist_fixed[sl],
                "wt2": wt2,
                "brow2": brow2,
                "wta": wta,
                "m32a": m32a,
            }
        )

    res = run_bass_kernel_spmd(nc, in_maps, core_ids=list(range(NCORES)), trace=trace)
    LAST_RESULT = res
    return np.concatenate([res.results[c]["out"] for c in range(NCORES)], axis=1)
